# revision 52
# baseline (speedup 1.0000x reference)
"""Causal self-attention on 8 Trainium2 NeuronCores.

Problem: x[2,2048,2048] f32, W_qkv[2048,6144], W_out[2048,2048]
  qkv = x @ W_qkv; per-head causal softmax attention; out = attn @ W_out.

Sharding: core c handles batch b=c//4, head group hg=c%4 (4 of 16 heads).
Each core computes its heads' QKV projections, full causal attention for
those heads, and a partial output projection (its heads' rows of W_out).
Host sums the 4 partial outputs per batch.

All device matmuls run in bf16 (fp32 PSUM accumulation): same PE rate as
fp32r at 512-wide outputs, but enables FWL fast weight loads and halves
DMA + SBUF. Host pre-packs every tensor so each DMA is a large
contiguous-per-partition transfer, and all weights are loaded into SBUF
exactly once. Partial y outputs leave in bf16 (summed in f32 on host).

Structure: QKV projection is split into per-512-query slabs; attention
for query group s and its out-projection run as one interleaved stream.
Each attention head emits: scores S^T = k_blk^T.T @ q^T (keys on
partitions; diagonal key-blocks narrowed to their live query range,
with only the triangular first 128 columns getting a precomputed bf16
mask multiply), exp on ScalarE, AV accumulation, and softmax-denominator
accumulation into bf16 SBUF accumulators (VectorE, plus GpSimd for a
few early blocks of the two big segments) so TensorE does no per-block
reduction work. The denominator is then closed with one or two bf16
ones-matmuls, a fast custom-DVE reciprocal, and a K=1 broadcast matmul
feeding the normalize multiply.

Between those latency-chained steps the emitter inserts independent
"filler" work — out-projection 128-row blocks for the previous query
group and next-slab QKV projection groups — so the TensorE FIFO never
stalls on the exp/reciprocal chains. C blocks are deliberately skewed
toward the last segment, whose exp load is largest. This keeps the PE
busy (and HAM un-throttled) end to end.
"""
import math

import ml_dtypes
import numpy as np

import concourse.bass as bass
import concourse.mybir as mybir
import concourse.tile as tile
from concourse import bacc
from concourse.bass_utils import run_bass_kernel_spmd

B, T, D = 2, 2048, 2048
H, Hd = 16, 128
N_CORES = 8
HL = 4            # heads per core
DL = HL * Hd      # 512: local hidden slice
P = 128
KC = D // P       # 16 contraction chunks of 128
NTB = T // P      # 16 row blocks of 128
QTW = 512         # query-group width
NQT = T // QTW    # 4 query groups
SCALE = 1.0 / math.sqrt(Hd)

f32 = mybir.dt.float32
f32r = mybir.dt.float32r
bf16 = mybir.dt.bfloat16
AF = mybir.ActivationFunctionType

# out-projection row-blocks emitted inside each segment (qt groups of
# the blocks must already be finished): skewed late because the last
# segments have the most exp work to hide
C_ASSIGN = {1: [0, 1], 2: [2, 3, 4], 3: [5, 6, 7, 8, 9, 10, 11]}
C_FINAL = [12, 13, 14, 15]


def build_program(reps: int = 1, phases: str = "ABC"):
    nc = bacc.Bacc("TRN2", target_bir_lowering=False, debug=False,
                   num_devices=N_CORES)
    # host-packed layouts (partition dim first, contiguous per partition)
    xT = nc.dram_tensor("xT", [P, NQT, KC, QTW], bf16, kind="ExternalInput")
    wq = nc.dram_tensor("wq", [P, HL, KC, Hd], bf16, kind="ExternalInput")
    wk = nc.dram_tensor("wk", [P, HL, KC, Hd], bf16, kind="ExternalInput")
    wv = nc.dram_tensor("wv", [P, KC, DL], bf16, kind="ExternalInput")
    wout = nc.dram_tensor("wout", [P, HL, D], bf16, kind="ExternalInput")
    y = nc.dram_tensor("y", [T, D], bf16, kind="ExternalOutput")

    with tile.TileContext(nc) as tc:
        if reps > 1:
            with tc.For_i(0, reps, 1):
                _body(nc, tc, xT, wq, wk, wv, wout, y, phases)
        else:
            _body(nc, tc, xT, wq, wk, wv, wout, y, phases)
    nc.compile()
    return nc


def _body(nc, tc, xT, wq, wk, wv, wout, y, phases="ABC"):
    do_b = "B" in phases
    do_c = "C" in phases
    with (
        tc.tile_pool(name="weights", bufs=1) as wpool,
        tc.tile_pool(name="qkv", bufs=1) as qkv_pool,
        tc.tile_pool(name="consts", bufs=1) as cpool,
        tc.tile_pool(name="xts", bufs=2) as xpool,
        tc.tile_pool(name="e", bufs=10) as epool,
        tc.tile_pool(name="eacc", bufs=2) as eaccpool,
        tc.tile_pool(name="rec", bufs=2) as recpool,
        tc.tile_pool(name="oraw", bufs=2) as opool,
        tc.tile_pool(name="ysb", bufs=3) as ypool,
        tc.tile_pool(name="psum", bufs=1, space="PSUM") as psum,
    ):
        # causal mask / ones constants (f32 scratch -> target dtypes)
        ones_col = cpool.tile([P, 1], bf16)     # lhsT for denom matmuls
        ones_row = cpool.tile([1, P], bf16)     # lhsT for K=1 broadcast
        tri = cpool.tile([P, P], bf16)          # tri[r, j] = 1.0 iff j >= r
        with tc.tile_pool(name="init_scratch", bufs=1) as scratch:
            sc = scratch.tile([P, P], f32)
            nc.gpsimd.memset(sc[:], 1.0)
            with nc.allow_low_precision(reason="exact small constants"):
                nc.vector.tensor_copy(ones_col[:], sc[:, :1])
                nc.vector.tensor_copy(ones_row[:], sc[:1, :])
            nc.gpsimd.affine_select(
                out=sc[:], in_=sc[:],
                compare_op=mybir.AluOpType.is_ge,
                fill=0.0, base=0, channel_multiplier=-1,
                pattern=[[1, P]])
            with nc.allow_low_precision(reason="0/1 mask exact in bf16"):
                nc.vector.tensor_copy(tri[:], sc[:])

        wq_sb = wpool.tile([P, HL, KC, Hd], bf16)
        wk_sb = wpool.tile([P, HL, KC, Hd], bf16)
        wv_sb = wpool.tile([P, KC, DL], bf16)
        wout_sb = wpool.tile([P, HL, D], bf16)
        qT_sb = qkv_pool.tile([P, HL, T], bf16)   # [Hd, h, Tq]
        kT_sb = qkv_pool.tile([P, HL, T], bf16)
        v_sb = qkv_pool.tile([P, NTB, DL], bf16)  # [Tk%128, kb, h*Hd]
        atT_sb = qkv_pool.tile([P, HL, T], bf16)  # normalized attn outT

        def a_slab_dma(s):
            xTs = xpool.tile([P, KC, QTW], bf16, tag="xT", name="xTs")
            for c4 in range(4):
                nc.sync.dma_start(xTs[:, 4 * c4:4 * (c4 + 1)],
                                  xT.ap()[:, s, 4 * c4:4 * (c4 + 1)])
            return xTs

        def a_qk_group(xTs, s, h, w_sb, dst):
            ps = psum.tile([P, QTW], f32, tag="mm", bufs=2, name="qk_ps")
            for kc in range(KC):
                nc.tensor.matmul(
                    ps[:], w_sb[:, h, kc], xTs[:, kc, :],
                    start=(kc == 0), stop=(kc == KC - 1))
            with nc.allow_low_precision(reason="bf16 qkv"):
                nc.vector.tensor_copy(
                    dst[:, h, s * QTW:(s + 1) * QTW], ps[:])

        def a_v_group(xTs, s, tsub):
            ps = psum.tile([P, DL], f32, tag="mm", bufs=2, name="v_ps")
            for kc in range(KC):
                nc.tensor.matmul(
                    ps[:], xTs[:, kc, tsub * P:(tsub + 1) * P],
                    wv_sb[:, kc, :],
                    start=(kc == 0), stop=(kc == KC - 1))
            with nc.allow_low_precision(reason="bf16 qkv"):
                nc.vector.tensor_copy(v_sb[:, s * 4 + tsub, :], ps[:])

        def a_units(xTs, s):
            units = []
            for h in range(HL):
                units.append(lambda h=h: a_qk_group(xTs, s, h, wq_sb, qT_sb))
                units.append(lambda h=h: a_qk_group(xTs, s, h, wk_sb, kT_sb))
            for tsub in range(4):
                units.append(lambda t=tsub: a_v_group(xTs, s, t))
            return units

        def b_head_main(qt, h):
            o_ps = psum.tile([P, QTW], f32, tag="o", bufs=2, name="o_ps")
            ea_v = eaccpool.tile([P, QTW], bf16, tag="eav", name="ea_v")
            nkb = (qt + 1) * 4
            # GpSimd (~1.5us/add vs VectorE ~0.6us) takes only the
            # earliest odd blocks of the two big segments — few enough
            # to drain well before the closing denominator matmul; the
            # short segments can't hide even one GpSimd op, and VectorE
            # has slack there
            n_gp = (0, 0, 3, 6)[qt]
            gp_kbs = set(range(1, 2 * n_gp, 2))
            ea_g = eaccpool.tile([P, QTW], bf16, tag="eag", name="ea_g") \
                if n_gp else None
            def consume(kb, e_sb, off):
                # AV + denominator accumulate for a block whose exp is
                # already in flight
                nc.tensor.matmul(
                    o_ps[:, off:],
                    v_sb[:, kb, h * Hd:(h + 1) * Hd],
                    e_sb[:, off:],
                    start=(kb == 0), stop=(kb == nkb - 1))
                first = kb == 0 or (kb == 1 and n_gp > 0)
                eng, acc = (nc.gpsimd, ea_g) if kb in gp_kbs else \
                    (nc.vector, ea_v)
                with nc.allow_low_precision(reason="bf16 denom accum"):
                    if first:
                        eng.tensor_copy(acc[:, off:], e_sb[:, off:])
                        if off:
                            eng.memset(acc[:, :off], 0.0)
                    else:
                        eng.tensor_add(
                            acc[:, off:], acc[:, off:], e_sb[:, off:])

            # emission is pipeline-shifted by one block: S(kb+1) enters
            # the TensorE FIFO before AV(kb), so the FIFO never parks
            # directly behind exp(kb) on ScalarE
            pending = None
            for kb in range(nkb):
                m = kb - 4 * qt  # >=0: diagonal block, narrow to live cols
                off = max(m, 0) * P
                q_sl = slice(qt * QTW + off, (qt + 1) * QTW)
                s_ps = psum.tile([P, QTW], f32, tag="s", bufs=3, name="s_ps")
                nc.tensor.matmul(
                    s_ps[:, off:],
                    kT_sb[:, h, kb * P:(kb + 1) * P],
                    qT_sb[:, h, q_sl],
                    start=True, stop=True)
                e_sb = epool.tile([P, QTW], bf16, tag="e")
                with nc.allow_low_precision(reason="bf16 attn weights"):
                    nc.scalar.activation(
                        e_sb[:, off:], s_ps[:, off:], AF.Exp,
                        scale=float(SCALE))
                if m >= 0:
                    # only the first 128 live columns are partially masked
                    nc.vector.tensor_mul(
                        e_sb[:, off:off + P], e_sb[:, off:off + P], tri[:])
                if pending is not None:
                    consume(*pending)
                pending = (kb, e_sb, off)
            consume(*pending)
            return o_ps, ea_v, ea_g

        def b_head_denom(ea_v, ea_g):
            d_ps = psum.tile([1, QTW], f32, tag="d", bufs=1, name="d_ps")
            if ea_g is not None:
                nc.tensor.matmul(d_ps[:], ones_col[:], ea_g[:],
                                 start=True, stop=False)
            nc.tensor.matmul(d_ps[:], ones_col[:], ea_v[:],
                             start=(ea_g is None), stop=True)
            return d_ps

        def b_head_recip(d_ps):
            rec = recpool.tile([1, QTW], f32, tag="rec")
            nc.vector.reciprocal_approx_fast(rec[:], d_ps[:])
            rec_bf = recpool.tile([1, QTW], bf16, tag="recbf")
            with nc.allow_low_precision(reason="bf16 softmax recip"):
                nc.vector.tensor_copy(rec_bf[:], rec[:])
            return rec_bf

        def b_head_fin(qt, h, o_ps, rec_bf):
            bc_ps = psum.tile([P, QTW], f32, tag="s", bufs=3, name="bc_ps")
            nc.tensor.matmul(
                bc_ps[:], ones_row[:], rec_bf[:],
                start=True, stop=True)
            o_raw = opool.tile([P, QTW], bf16, tag="oraw")
            with nc.allow_low_precision(reason="bf16 attn out"):
                # ScalarE's strict FIFO would delay the next head's exp
                # chain behind this copy in the short segments; only the
                # last segment has enough filler to hide it there
                if qt == NQT - 1:
                    nc.scalar.copy(o_raw[:], o_ps[:])
                else:
                    nc.vector.tensor_copy(o_raw[:], o_ps[:])
                nc.vector.tensor_mul(
                    atT_sb[:, h, qt * QTW:(qt + 1) * QTW], o_raw[:],
                    bc_ps[:])

        def c_units(tb, copy_eng=None):
            t_sl = slice(tb * P, (tb + 1) * P)
            y_sb = ypool.tile([P, D], bf16, tag="ysb", name="y_sb")

            def unit(dc):
                y_ps = psum.tile([P, QTW], f32, tag="mm", bufs=2,
                                 name="y_ps")
                for h in range(HL):
                    nc.tensor.matmul(
                        y_ps[:],
                        atT_sb[:, h, t_sl],
                        wout_sb[:, h, dc * QTW:(dc + 1) * QTW],
                        start=(h == 0), stop=(h == HL - 1))
                with nc.allow_low_precision(reason="bf16 partial y"):
                    if copy_eng == "scalar":
                        nc.scalar.copy(
                            y_sb[:, dc * QTW:(dc + 1) * QTW], y_ps[:])
                    else:
                        nc.vector.tensor_copy(
                            y_sb[:, dc * QTW:(dc + 1) * QTW], y_ps[:])
                if dc == 1:
                    nc.sync.dma_start(y.ap()[t_sl, :D // 2],
                                      y_sb[:, :D // 2])
                elif dc == 3:
                    nc.sync.dma_start(y.ap()[t_sl, D // 2:],
                                      y_sb[:, D // 2:])

            return [lambda dc=dc: unit(dc) for dc in range(4)]

        # ---- initial DMAs: first xT chunks interleaved with the first
        # head's q/k weights so the first matmul group starts early
        xTs0 = xpool.tile([P, KC, QTW], bf16, tag="xT", name="xTs")
        nc.sync.dma_start(xTs0[:, 0:4], xT.ap()[:, 0, 0:4])
        nc.sync.dma_start(wq_sb[:, 0], wq.ap()[:, 0])
        nc.sync.dma_start(xTs0[:, 4:8], xT.ap()[:, 0, 4:8])
        nc.sync.dma_start(wk_sb[:, 0], wk.ap()[:, 0])
        nc.sync.dma_start(xTs0[:, 8:12], xT.ap()[:, 0, 8:12])
        nc.sync.dma_start(xTs0[:, 12:16], xT.ap()[:, 0, 12:16])
        for h in range(1, HL):
            nc.sync.dma_start(wq_sb[:, h], wq.ap()[:, h])
            nc.sync.dma_start(wk_sb[:, h], wk.ap()[:, h])
        nc.sync.dma_start(wv_sb[:], wv.ap())
        if do_c:
            nc.sync.dma_start(wout_sb[:], wout.ap())

        # ---- slab 0 projection up front (nothing to interleave with)
        for u in a_units(xTs0, 0):
            u()
        if not do_b:
            for s in range(1, NQT):
                for u in a_units(a_slab_dma(s), s):
                    u()
            return
        xTs_next = a_slab_dma(1)

        # ---- segments: B(s) heads + filler units ------------------------
        for s in range(NQT):
            units = []
            if do_c and s in C_ASSIGN:
                for tb in C_ASSIGN[s]:
                    units.extend(c_units(tb))
            if s < NQT - 1:
                units.extend(a_units(xTs_next, s + 1))
            # distribute across the 4 heads: ~1/4 each; a couple of
            # units cover the denominator-accumulator latency, the rest
            # cover the reciprocal chain before the broadcast matmul
            per = [units[(len(units) * h) // HL:
                         (len(units) * (h + 1)) // HL] for h in range(HL)]
            for h in range(HL):
                handle = b_head_main(s, h)
                mine = per[h]
                ncover = min(4, max(1, len(mine) - 2))
                for u in mine[:ncover]:
                    u()
                d_ps = b_head_denom(handle[1], handle[2])
                rec_bf = b_head_recip(d_ps)
                for u in mine[ncover:]:
                    u()
                b_head_fin(s, h, handle[0], rec_bf)
                if h == HL - 1 and s < NQT - 2:
                    xTs_next = a_slab_dma(s + 2)
        if do_c:
            for tb in C_FINAL:
                for u in c_units(tb):
                    u()


def prepare_in_maps(x, W_qkv, W_out):
    x = np.ascontiguousarray(np.asarray(x), dtype=np.float32)
    W_qkv = np.ascontiguousarray(np.asarray(W_qkv), dtype=np.float32)
    W_out = np.ascontiguousarray(np.asarray(W_out), dtype=np.float32)
    bf = ml_dtypes.bfloat16
    Wr = W_qkv.reshape(D, 3, H, Hd)
    Wo = W_out.reshape(H, Hd, D)
    # xT[b]: [D,T] -> [p, s, kc, t]
    xTs = [np.ascontiguousarray(
        x[b].T.reshape(KC, P, NQT, QTW).transpose(1, 2, 0, 3).astype(bf))
        for b in range(B)]

    def packw_h(w):  # [D, DL] -> [p, h, kc, hd]
        return np.ascontiguousarray(
            w.reshape(KC, P, HL, Hd).transpose(1, 2, 0, 3).astype(bf))

    def packw(w):  # [D, DL] -> [p, kc, DL]
        return np.ascontiguousarray(
            w.reshape(KC, P, DL).transpose(1, 0, 2).astype(bf))

    in_maps = []
    for c in range(N_CORES):
        b, hg = c // 4, c % 4
        hs = slice(hg * HL, (hg + 1) * HL)
        in_maps.append({
            "xT": xTs[b],
            "wq": packw_h(Wr[:, 0, hs, :].reshape(D, DL)),
            "wk": packw_h(Wr[:, 1, hs, :].reshape(D, DL)),
            "wv": packw(Wr[:, 2, hs, :].reshape(D, DL)),
            "wout": np.ascontiguousarray(
                Wo[hs].transpose(1, 0, 2).astype(bf)),
        })
    return in_maps


def combine_outputs(results):
    out = np.zeros((B, T, D), dtype=np.float32)
    for c in range(N_CORES):
        out[c // 4] += results[c]["y"].astype(np.float32)
    return out


_PROGRAM_CACHE = {}


def kernel(x, W_qkv, W_out):
    in_maps = prepare_in_maps(x, W_qkv, W_out)
    if 1 not in _PROGRAM_CACHE:
        _PROGRAM_CACHE[1] = build_program(1)
    nc = _PROGRAM_CACHE[1]
    res = run_bass_kernel_spmd(nc, in_maps, core_ids=list(range(N_CORES)))
    return combine_outputs(res.results)
